# revision 28
# baseline (speedup 1.0000x reference)
"""Trainium2 Bass kernel for GCN(2-layer) -> sum-pool -> LSTM(2-layer) -> classifier -> softmax.

Strategy:
  - Data-parallel: shard batch B=16 across 8 cores (B'=2 each); host
    concatenates the per-core [2,10] outputs.
  - All weights are uploaded REPLICATED (one [128, WK] bf16 column-pack per
    core) and DMA'd straight from HBM; x is uploaded bf16 host-transposed to
    [N, T, B', F]. The graded metric is per-core device time, so upload bytes
    are free.
  - GCN scatter-add == multiply by a fixed normalized adjacency A [128,128]
    (built on host from edge_index). Per (t,b) pair the chain alternates
    orientation so every product is a plain matmul with no transposes:
      u1:  pz   = (A@X)^T       [f on partitions]   (lhsT=x chunks, rhs=A^T)
      u2a: ztb  = copy(pz)                           DVE
      u2b: py1T = (A@X@W1)^T + 1(x)b1 [f1 on parts] (lhsT=W1 blocks, rhs=ztb,
                                                     b1 via rank-1 PE matmul)
      u3a: h1T  = relu(py1T)    one [128,512] ACT op
      u4:  py2n = h1@W2         [N on partitions]   (lhsT=h1T chunks, rhs=W2)
      u4a: p2b  = copy(py2n)                         DVE
      u5:  pyA  = A@(h1@W2) + 1(x)b2 [N on parts]   (lhsT=A^T, rhs=p2b -- the
                                                     PRESERVE orientation)
      u5b: h2n  = relu(pyA)     one [128,256] ACT op
      seq-pool: node-sum via PE ones-column matmuls into spare PSUM columns
                of the psm ring (nearly free), then one tiny DVE cast (u6).
    Stages are hand-software-pipelined with a 1-tick skew; psum-reading
    stages are emitted before psum-allocating ones each tick so buffer
    recycling WAR waits are pre-covered.
  - The LSTM is MERGED into the GCN pipeline: seq col t pools at tick t+9,
    its L0 input projection fires at tick t+10, and LSTM slot t runs at tick
    t+11 overlapping remaining GCN work.
  - LSTM slots process BOTH layers with combined ops: layer-1's gates for
    step t live at slot position t+2 of the gate psum, so slot s reads one
    contiguous [128, (layer,gate,b)] block: ONE 32-col tanh for both cells,
    one op each for u/v/c/tanh(c)/h over [128,8] both-layer tiles. The gate
    psum is two half-T [128, (th,l,j,b)] banks; slot positions wrap mod 32 so
    layer-1 steps 30/31 reuse the (never-read) slot-0/1 layer-1 positions,
    whose preloaded bias is exactly the right initial value.
  - LSTM in gates-transposed orientation, weights stationary. sigma(x) =
    (tanh(x/2)+1)/2 with the 1/2 pre-folded into weight rows, hidden state
    carried as H = 2h (folded into Whh/Wih1/Wc). One ACT table set.
  - Classifier + softmax (exp with fused accum_out sum) on device.

Environment quirks handled:
  - This walrus build supports a single sync-wait command per instruction;
    a TileContext monkeypatch legalizes Tile's multi-wait instructions by
    moving extra waits onto same-engine NoOps (_install_single_wait_legalizer).
  - DMA instructions cannot carry WAR waits at all, so x is preloaded into
    SBUF once via fresh-buffer DMAs and weight/x pools are never released.
  - GPSIMD/Pool cannot access PSUM and fails the ISA check for
    TensorScalarPtr; Pool is used only as a second DMA-trigger queue.
"""

import numpy as np

T, B, N, F_IN = 32, 16, 128, 256
EMB, HID, NCLS = 128, 256, 10
NCORES = 8
BSH = B // NCORES  # 2
NSTEP = T
TB = T * BSH  # 64 (t,b) units per core

# Column blocks of the packed weight tensor [128, WK] (bf16), replicated to
# every core.
WCOLS = {
    "atw": (0, 128),
    "w1": (128, 640),
    "w2": (640, 896),
    "wc": (896, 936),
    "wih0": (936, 1960),
    "whh0": (1960, 4008),
    "wih1": (4008, 6056),
    "whh1": (6056, 8104),
}
WK = 8104

_CACHE = {}


def _f32(x):
    return np.ascontiguousarray(np.asarray(x, dtype=np.float32))


def _bf16(x):
    import ml_dtypes

    return np.ascontiguousarray(np.asarray(x, dtype=np.float32).astype(ml_dtypes.bfloat16))


def _host_adjacency(edge_index):
    """Dense normalized adjacency with self loops; returns A^T [N,N] fp32."""
    ei = np.asarray(edge_index, dtype=np.int64)
    loop = np.arange(N, dtype=np.int64)
    src = np.concatenate([ei[0], loop])
    dst = np.concatenate([ei[1], loop])
    deg = np.zeros((N,), np.float32)
    np.add.at(deg, dst, np.float32(1.0))
    dinv = deg.astype(np.float32) ** -0.5
    norm = (dinv[src] * dinv[dst]).astype(np.float32)
    A = np.zeros((N, N), np.float32)
    np.add.at(A, (dst, src), norm)
    return A.T.copy()


def _host_weights(inp):
    """Prepare all device weight tensors (bf16) from raw inputs."""
    W1 = _f32(inp["W1"])
    b1 = _f32(inp["b1"])
    W2 = _f32(inp["W2"])
    b2 = _f32(inp["b2"])
    Wc = _f32(inp["Wc"])
    bc = _f32(inp["bc"])

    # gate permutation (i,f,g,o) -> (i,f,o,g), and sigma-via-tanh row scaling
    perm = np.concatenate(
        [np.arange(0, 512), np.arange(768, 1024), np.arange(512, 768)]
    )
    srow = np.ones((4 * HID,), np.float32)
    srow[: 3 * HID] = 0.5  # i,f,o rows (after permutation)

    def prep_lstm(Wih, Whh, bih, bhh, in_scale):
        Wih = _f32(Wih)[perm] * srow[:, None] * in_scale
        Whh = _f32(Whh)[perm] * srow[:, None] * 0.5  # H = 2h convention
        bb = (_f32(bih) + _f32(bhh))[perm] * srow
        return Wih, Whh, bb

    Wih0p, Whh0p, b0p = prep_lstm(inp["Wih0"], inp["Whh0"], inp["bih0"], inp["bhh0"], 1.0)
    Wih1p, Whh1p, b1lp = prep_lstm(inp["Wih1"], inp["Whh1"], inp["bih1"], inp["bhh1"], 0.5)
    Wcp = Wc * 0.5

    def lhsT_chunks(Wp, kchunks):
        # Wp [4H, K]; device layout [128, kchunks*8*128]:
        # dev[:, (kc*8+jc)*128 : +128] = Wp[jc-block, kc-block].T
        M4, K = Wp.shape
        assert M4 == 4 * HID and K == kchunks * 128
        return (
            Wp.reshape(8, 128, kchunks, 128).transpose(3, 2, 0, 1).reshape(128, kchunks * 8 * 128)
        )

    import ml_dtypes

    wpack = np.zeros((128, WK), dtype=ml_dtypes.bfloat16)

    def put(key, arr):
        c0, c1 = WCOLS[key]
        wpack[:, c0:c1] = _bf16(arr)

    put("atw", _host_adjacency(inp["edge_index"]))
    # w1 block layout [f_in_p, (kc, mc, f_out)]: lhsT slice (kc,mc) multiplies
    # ztb chunk kc into py1T chunk mc.
    put("w1", W1.reshape(2, 128, 2, 128).transpose(1, 0, 2, 3).reshape(128, 512))
    # w2 block layout [f1_p, (kc, EMB)]
    put("w2", W2.reshape(2, 128, EMB).transpose(1, 0, 2).reshape(128, 2 * EMB))
    put("wih0", lhsT_chunks(Wih0p, 1))
    put("whh0", lhsT_chunks(Whh0p, 2))
    put("wih1", lhsT_chunks(Wih1p, 2))
    put("whh1", lhsT_chunks(Whh1p, 2))
    put("wc", Wcp.reshape(4, 128, NCLS).transpose(1, 0, 2).reshape(128, 4 * NCLS))

    # single partition row: rank-1 matmul lhsT operands need base partition 0
    vpack = np.zeros((1, 4096), dtype=ml_dtypes.bfloat16)
    vpack[0, 0:1024] = _bf16(b0p)
    vpack[0, 1024:2048] = _bf16(b1lp)
    vpack[0, 2048:2304] = _bf16(b1)                      # b1row (rank-1 lhsT)
    vpack[0, 2304:2560] = _bf16(np.concatenate([b2, b2]))  # b2row2 (rank-1 rhs)
    vpack[0, 2560 : 2560 + NCLS] = _bf16(bc)

    return {
        "wpack": np.ascontiguousarray(wpack),
        "vpack": vpack,
    }


def _install_single_wait_legalizer():
    """This environment's walrus build supports exactly ONE sync-wait command
    per instruction (setupSyncWait 'Too many sync wait commands'). Tile freely
    emits 2+ waits. Legalize: extra waits move onto same-engine NoOps inserted
    immediately before the instruction (engines dispatch in order, so the
    blocking semantics are identical)."""
    import concourse.tile as tile
    from concourse import mybir

    if getattr(tile.TileContext, "_single_wait_patched", False):
        return

    _orig_commit = tile.TileContext._commit_instruction

    def _patched_commit(self, inst, lazy_reg_writes=True):
        si = inst.sync_info
        if (
            si is not None
            and si.on_wait
            and len(si.on_wait) > 1
            and inst.engine != mybir.EngineType.Unassigned
        ):
            waits = list(si.on_wait)
            inst.sync_info = mybir.SyncInfo(
                on_wait=[waits[-1]], on_update=list(si.on_update)
            )
            for w in waits[:-1]:
                nop = mybir.InstNoOp(
                    name=self.nc.get_next_instruction_name(),
                    engine=inst.engine,
                    sync_info=mybir.SyncInfo(on_wait=[w], on_update=[]),
                )
                self._add_instruction(nop)
        return _orig_commit(self, inst, lazy_reg_writes)

    _orig_dab = tile.TileContext._drain_and_barrier

    def _patched_dab(self, tick_clock, wait_clock):
        from concourse.vector_clock import ScopedClock

        pre = self.nc.sync.nop(nofuse=True)
        wait_clock.add_sem_waits(
            pre.ins, ScopedClock({None: tick_clock.global_clock})
        )
        si = pre.ins.sync_info
        if si is not None and si.on_wait and len(si.on_wait) > 1:
            waits = list(si.on_wait)
            pre.ins.sync_info = mybir.SyncInfo(
                on_wait=[waits[0]], on_update=list(si.on_update)
            )
            for w in waits[1:]:
                n2 = self.nc.sync.nop(nofuse=True)
                n2.ins.sync_info = mybir.SyncInfo(on_wait=[w], on_update=[])
        ret = _orig_dab(self, tick_clock, wait_clock)
        for i in self.nc.cur_bb.bb.instructions:
            si2 = i.sync_info
            if si2 is not None and si2.on_wait and len(si2.on_wait) > 1:
                i.sync_info = mybir.SyncInfo(
                    on_wait=[si2.on_wait[0]], on_update=list(si2.on_update)
                )
        return ret

    tile.TileContext._commit_instruction = _patched_commit
    tile.TileContext._drain_and_barrier = _patched_dab
    tile.TileContext._single_wait_patched = True


def build_program():
    import concourse.bass as bass
    import concourse.tile as tile
    from concourse import mybir
    from contextlib import ExitStack

    _install_single_wait_legalizer()

    dt = mybir.dt
    AF = mybir.ActivationFunctionType
    OP = mybir.AluOpType

    nc = bass.Bass("TRN2", target_bir_lowering=False, debug=False, num_devices=NCORES)

    # ---- dram tensors ----
    x_d = nc.dram_tensor("x", [N, T, BSH, F_IN], dt.bfloat16, kind="ExternalInput").ap()
    wpack_d = nc.dram_tensor("wpack", [128, WK], dt.bfloat16, kind="ExternalInput").ap()
    vpack_d = nc.dram_tensor("vpack", [1, 4096], dt.bfloat16, kind="ExternalInput").ap()
    out_d = nc.dram_tensor("out", [BSH, NCLS], dt.float32, kind="ExternalOutput").ap()

    with tile.TileContext(nc) as tc, ExitStack() as ctx:
        # persistent state buffers
        spool = ctx.enter_context(tc.tile_pool(name="state", bufs=1))
        seqT = spool.tile([128, TB], dt.bfloat16, tag="seqT")
        zh = spool.tile([128, 2 * BSH], dt.bfloat16, tag="zh")  # zero H
        zc8 = spool.tile([128, 4 * BSH], dt.float32, tag="zc8")  # zero c (both layers)
        nc.vector.memset(zh[:], 0.0)
        nc.vector.memset(zc8[:], 0.0)

        # x fully preloaded into SBUF with fresh-buffer DMAs (single-wait DMA
        # restriction). Host-transposed to [N,T,B',F]: contiguous copies.
        xall = spool.tile([128, TB * F_IN], dt.bfloat16, tag="xall")
        xav = xall[:].rearrange("n (t b f) -> n t b f", t=T, b=BSH, f=F_IN)

        # ---- persistent sbuf: weights ----
        wpool = ctx.enter_context(tc.tile_pool(name="weights", bufs=1))
        ws = {}

        def weight_dma(k, eng=None):
            eng = eng or nc.sync
            c0, c1 = WCOLS[k]
            ws[k] = wpool.tile([128, c1 - c0], dt.bfloat16, tag=k, name=f"w_{k}")
            eng.dma_start(ws[k][:], wpack_d[:, c0:c1])

        weight_dma("atw")
        for tch in range(8):
            nc.sync.dma_start(
                xav[:, 4 * tch : 4 * tch + 4],
                x_d[:, 4 * tch : 4 * tch + 4],
            )
            if tch == 0:
                weight_dma("w1")
            elif tch == 1:
                weight_dma("w2")
            elif tch == 2:
                weight_dma("wc")
        # bias rows: one [1,4096] DMA at the FRONT of the Pool queue; the
        # bias-prefill matmuls are emitted at tick 8 of the loop (not before
        # it) so the in-order PE stream never blocks on this DMA.
        vpt = wpool.tile([1, 4096], dt.bfloat16, tag="vpt", name="w_vpt")
        nc.gpsimd.dma_start(vpt[:], vpack_d)
        for k in ["wih0", "whh0", "wih1", "whh1"]:
            weight_dma(k, eng=nc.gpsimd)
        ws["b0row"] = vpt[0:1, 0 : 4 * HID]
        ws["b1lrow"] = vpt[0:1, 1024 : 1024 + 4 * HID]
        ws["b1row"] = vpt[0:1, 2048:2304]
        ws["b2row2"] = vpt[0:1, 2304:2560]
        ws["bcrow"] = vpt[0:1, 2560 : 2560 + NCLS]
        onesrow = wpool.tile([1, 256], dt.bfloat16, tag="onesrow", name="w_onesrow")
        nc.vector.memset(onesrow[:], 1.0)
        onescol = wpool.tile([128, 1], dt.bfloat16, tag="onescol", name="w_onescol")
        nc.vector.memset(onescol[:], 1.0)

        # ---- LSTM gate psums: two half-T banks laid out [th, layer, j, b];
        # slot s (L0 step s, L1 step s-2) reads one contiguous 32-col block.
        # Slot positions wrap mod 32: L1 steps 30/31 land on the layer-1
        # halves of slots 0/1 (bias prefilled there is the correct init).
        lpool = ctx.enter_context(tc.tile_pool(name="lstm", bufs=4))
        pg_pool = ctx.enter_context(tc.tile_pool(name="pgates", bufs=1, space="PSUM"))
        # layout [p, (l, j, th, b)]: bias prefill per (l,j) is a contiguous
        # 32-col write; per-slot gate MMs hit contiguous [128,2] blocks; the
        # slot tanh reads a strided (l, j, b) block at fixed th (ACT handles
        # strided APs at the same cost).
        pgA = pg_pool.tile([128, 512], dt.float32, tag="pgA")
        pgB = pg_pool.tile([128, 512], dt.float32, tag="pgB")
        pgAv = pgA[:].rearrange("p (l j th b) -> p l j th b", th=16, l=2, j=8, b=BSH)
        pgBv = pgB[:].rearrange("p (l j th b) -> p l j th b", th=16, l=2, j=8, b=BSH)

        def pgslot(s):
            s = s % 32
            return (pgAv, s) if s < 16 else (pgBv, s - 16)

        def pg_bias_prefill():
            # opens each bank's accumulation group (emitted at loop tick 8:
            # after the vpt DMA has landed, before any gate matmul)
            for pgt in (pgA, pgB):
                first = True
                for l, row in ((0, "b0row"), (1, "b1lrow")):
                    for jc in range(8):
                        nc.tensor.matmul(
                            pgt[:, (l * 8 + jc) * 32 : (l * 8 + jc + 1) * 32],
                            ws[row][:, jc * 128 : (jc + 1) * 128],
                            onesrow[:, 0 : 2 * 16],
                            start=first,
                            stop=False,
                            skip_group_check=True,
                        )
                        first = False

        # ---- LSTM slot machinery ----
        h_tiles = {}   # slot -> combined h tile [128, (l, hc, b)]
        c_tiles = {}   # slot -> combined c tile [128, (l, hc, b)] f32

        def l0_mms(t):
            vv, th = pgslot(t)
            for jc in range(8):
                for kc in range(2):
                    rhs = (
                        zh[:, kc * BSH : (kc + 1) * BSH]
                        if t == 0
                        else h_tiles[t - 1][:, kc * BSH : (kc + 1) * BSH]
                    )
                    nc.tensor.matmul(
                        vv[:, 0, jc, th, :],
                        ws["whh0"][:, (kc * 8 + jc) * 128 : (kc * 8 + jc + 1) * 128],
                        rhs,
                        start=False,
                        stop=(t == NSTEP - 1 and jc == 7 and kc == 1),
                        skip_group_check=True,
                    )

        def l1_whh(t):
            vv, th = pgslot(t + 2)
            for jc in range(8):
                for kc in range(2):
                    rhs = (
                        zh[:, kc * BSH : (kc + 1) * BSH]
                        if t == 0
                        else h_tiles[(t - 1) + 2][:, 4 + kc * BSH : 4 + (kc + 1) * BSH]
                    )
                    nc.tensor.matmul(
                        vv[:, 1, jc, th, :],
                        ws["whh1"][:, (kc * 8 + jc) * 128 : (kc * 8 + jc + 1) * 128],
                        rhs,
                        start=False,
                        stop=(t == NSTEP - 1 and jc == 7 and kc == 1),
                        skip_group_check=True,
                    )

        def l1_wih(t):
            # input projection Wih1 @ y0_t, pre-emitted one slot early (its
            # input h_tiles[t][:, L0 half] is already a slot old)
            vv, th = pgslot(t + 2)
            for jc in range(8):
                for kc in range(2):
                    nc.tensor.matmul(
                        vv[:, 1, jc, th, :],
                        ws["wih1"][:, (kc * 8 + jc) * 128 : (kc * 8 + jc + 1) * 128],
                        h_tiles[t][:, kc * BSH : (kc + 1) * BSH],
                        start=False,
                        stop=False,
                        skip_group_check=True,
                    )

        def l0_proj(t):
            vv, th = pgslot(t)
            for jc in range(8):
                nc.tensor.matmul(
                    vv[:, 0, jc, th, :],
                    ws["wih0"][:, jc * 128 : (jc + 1) * 128],
                    seqT[:, BSH * t : BSH * (t + 1)],
                    start=False,
                    stop=False,
                    skip_group_check=True,
                )

        sstate = {}

        def lstm_slot_a(s):
            # gate MMs this slot still owes (L1 Whh for step s-2, L0 Whh for
            # step s), then the first half of the combined both-layer cell
            # (tanh + u/v/c). The second half (tc + h) is emitted AFTER the
            # tick's GCN ACT/DVE work so those ops fill the chain's sem-wait
            # gaps instead of head-of-line blocking behind tc.
            if 2 <= s:
                l1_whh(s - 2)
            if s < NSTEP:
                l0_mms(s)
            vv, th = pgslot(s)
            tt = lpool.tile([128, 32], dt.float32, tag="tt", name=f"tt_{s}")
            ttv = tt[:].rearrange("p (l j b) -> p l j b", l=2, j=8, b=BSH)
            nc.scalar.activation(ttv, vv[:, :, :, th, :], AF.Tanh)
            ti = ttv[:, :, 0:2, :]
            tf = ttv[:, :, 2:4, :]
            tg = ttv[:, :, 6:8, :]
            u = lpool.tile([128, 8], dt.float32, tag="u", name=f"u_{s}")
            uv = u[:].rearrange("p (l hc b) -> p l hc b", l=2, hc=2, b=BSH)
            nc.vector.scalar_tensor_tensor(uv, ti, 1.0, tg, OP.add, OP.mult)
            v = lpool.tile([128, 8], dt.float32, tag="v", name=f"v_{s}")
            vvw = v[:].rearrange("p (l hc b) -> p l hc b", l=2, hc=2, b=BSH)
            if s == 0:
                cp = zc8[:].rearrange("p (l hc b) -> p l hc b", l=2, hc=2, b=BSH)
                nc.vector.scalar_tensor_tensor(vvw, tf, 1.0, cp, OP.add, OP.mult)
            elif s == 2:
                # L1 half restarts (step 0): its c_prev is zero
                cpl0 = c_tiles[1][:].rearrange("p (l hc b) -> p l hc b", l=2, hc=2, b=BSH)
                nc.vector.scalar_tensor_tensor(
                    vvw[:, 0], tf[:, 0], 1.0, cpl0[:, 0], OP.add, OP.mult
                )
                zcv = zc8[:].rearrange("p (l hc b) -> p l hc b", l=2, hc=2, b=BSH)
                nc.vector.scalar_tensor_tensor(
                    vvw[:, 1], tf[:, 1], 1.0, zcv[:, 1], OP.add, OP.mult
                )
            else:
                cp = c_tiles[s - 1][:].rearrange("p (l hc b) -> p l hc b", l=2, hc=2, b=BSH)
                nc.vector.scalar_tensor_tensor(vvw, tf, 1.0, cp, OP.add, OP.mult)
            c_new = lpool.tile([128, 8], dt.float32, tag="c", name=f"c_{s}")
            nc.vector.scalar_tensor_tensor(c_new[:], v[:], 0.5, u[:], OP.mult, OP.add)
            c_tiles[s] = c_new
            sstate[s] = ttv

        def lstm_slot_b1(s):
            # tc only: emitted after a first chunk of GCN ACT fill so the ACT
            # queue reaches it right around when c's semaphore fires
            c_new = c_tiles[s]
            tc_ = lpool.tile([128, 8], dt.float32, tag="tc", name=f"tc_{s}")
            nc.scalar.activation(tc_[:], c_new[:], AF.Tanh, scale=0.5)
            sstate[s] = (sstate.pop(s), tc_)

        def lstm_slot_b2(s):
            ttv, tc_ = sstate.pop(s)
            to = ttv[:, :, 4:6, :]
            h = lpool.tile([128, 8], dt.bfloat16, tag="h", name=f"h_{s}")
            hv = h[:].rearrange("p (l hc b) -> p l hc b", l=2, hc=2, b=BSH)
            tcv = tc_[:].rearrange("p (l hc b) -> p l hc b", l=2, hc=2, b=BSH)
            nc.vector.scalar_tensor_tensor(hv, to, 1.0, tcv, OP.add, OP.mult)
            h_tiles[s] = h
            # pre-emit L1's input projection for step s-1 (consumed at s+1)
            if 1 <= s <= NSTEP:
                l1_wih(s - 1)

        # ================= merged GCN + LSTM pipeline =================
        with (
            tc.tile_pool(name="interm", bufs=3) as ipool,
            tc.tile_pool(name="pzp", bufs=2, space="PSUM") as pzpool,
            tc.tile_pool(name="py1p", bufs=2, space="PSUM") as py1pool,
            tc.tile_pool(name="psmall", bufs=2, space="PSUM") as pspool,
        ):
            npair = T * BSH // 2
            st = [dict() for _ in range(npair)]

            def u1(p):
                pz = pzpool.tile([128, 4 * N], dt.float32, tag="pz", bufs=2, name=f"pz{p}")
                for u in range(2):
                    xb = xall[:, (2 * p + u) * F_IN : (2 * p + u + 1) * F_IN]
                    for kc in range(2):
                        nc.tensor.matmul(
                            pz[:, (2 * u + kc) * N : (2 * u + kc + 1) * N],
                            xb[:, kc * 128 : (kc + 1) * 128],
                            ws["atw"][:],
                            start=(u == 0 and kc == 0),
                            stop=(u == 1 and kc == 1),
                            skip_group_check=True,
                        )
                st[p]["pz"] = pz

            def u2a(p):
                pz = st[p].pop("pz")
                ztb = ipool.tile([128, 4 * N], dt.bfloat16, tag="ztb", name=f"ztb{p}")
                nc.vector.tensor_copy(ztb[:], pz[:])
                st[p]["ztb"] = ztb

            def u2b(p):
                # py1T = (A@X@W1)^T blocks [128, (mc, u, N)] + rank-1 b1
                ztb = st[p].pop("ztb")
                py1 = py1pool.tile([128, 4 * N], dt.float32, tag="py1", bufs=2, name=f"py1_{p}")
                first = True
                for mc in range(2):
                    for u in range(2):
                        for kc in range(2):
                            nc.tensor.matmul(
                                py1[:, (2 * mc + u) * N : (2 * mc + u + 1) * N],
                                ws["w1"][:, (kc * 2 + mc) * 128 : (kc * 2 + mc + 1) * 128],
                                ztb[:, (2 * u + kc) * N : (2 * u + kc + 1) * N],
                                start=first,
                                stop=False,
                                skip_group_check=True,
                            )
                            first = False
                    nc.tensor.matmul(
                        py1[:, 2 * mc * N : 2 * (mc + 1) * N],
                        ws["b1row"][:, mc * 128 : (mc + 1) * 128],
                        onesrow[:, 0 : 2 * N],
                        start=False,
                        stop=(mc == 1),
                        skip_group_check=True,
                    )
                st[p]["py1"] = py1

            def u3a(p):
                py1 = st[p].pop("py1")
                h1 = ipool.tile([128, 4 * N], dt.bfloat16, tag="h1", name=f"h1_{p}")
                nc.scalar.activation(h1[:], py1[:], AF.Relu)
                st[p]["h1"] = h1

            def u4(p):
                # py2n = h1@W2 [N, (u, EMB)]
                h1 = st[p].pop("h1")
                py2 = pspool.tile(
                    [128, 2 * EMB + 4], dt.float32, tag="psm", bufs=2, name=f"py2_{p}"
                )
                first = True
                for u in range(2):
                    for mc in range(2):
                        nc.tensor.matmul(
                            py2[:, u * EMB : (u + 1) * EMB],
                            h1[:, (2 * mc + u) * N : (2 * mc + u + 1) * N],
                            ws["w2"][:, mc * EMB : (mc + 1) * EMB],
                            start=first,
                            stop=(u == 1 and mc == 1),
                            skip_group_check=True,
                        )
                        first = False
                st[p]["py2"] = py2

            def u4a(p):
                py2 = st[p].pop("py2")
                p2b = ipool.tile([128, 2 * EMB], dt.bfloat16, tag="p2b", name=f"p2b_{p}")
                nc.vector.tensor_copy(p2b[:], py2[:, 0 : 2 * EMB])
                st[p]["p2b"] = p2b

            def u5(p):
                # pyA = A @ (h1@W2) [N, (u,EMB)] (preserve form) + rank-1 b2;
                # spare cols 256:260 of this bank take pair p-2's seq-pool.
                p2b = st[p].pop("p2b")
                pyA = pspool.tile(
                    [128, 2 * EMB + 4], dt.float32, tag="psm", bufs=2, name=f"pyA_{p}"
                )
                nc.tensor.matmul(
                    pyA[:, 0 : 2 * EMB],
                    ws["atw"][:],
                    p2b[:],
                    start=True,
                    stop=False,
                    skip_group_check=True,
                )
                nc.tensor.matmul(
                    pyA[:, 0 : 2 * EMB],
                    onesrow[:, 0:128],
                    ws["b2row2"][:],
                    start=False,
                    stop=False,
                    skip_group_check=True,
                )
                st[p]["pyA"] = pyA
                seq_pool(p - 2, pyA, final_stop=True)

            def seq_pool(p, tile_, final_stop):
                # node-sum of pair p via PE ones-column matmuls into psum
                # spare cols; nearly free on PE.
                if not (0 <= p < npair):
                    if final_stop:
                        # close the accumulation group without seq writes
                        nc.tensor.matmul(
                            tile_[0:1, 2 * EMB : 2 * EMB + 1],
                            onescol[:],
                            onescol[:],
                            start=False,
                            stop=True,
                            skip_group_check=True,
                        )
                    return
                h2n = st[p].pop("h2n")
                for u in range(2):
                    nc.tensor.matmul(
                        tile_[:, 2 * EMB + u : 2 * EMB + u + 1],
                        h2n[:, u * EMB : (u + 1) * EMB],
                        onescol[:],
                        start=False,
                        stop=(final_stop and u == 1),
                        skip_group_check=True,
                    )
                st[p]["seqtile"] = tile_

            def u5b(p):
                pyA = st[p].pop("pyA")
                h2n = ipool.tile([128, 2 * EMB], dt.bfloat16, tag="h2", name=f"h2_{p}")
                nc.scalar.activation(h2n[:], pyA[:, 0 : 2 * EMB], AF.Relu)
                st[p]["h2n"] = h2n

            def u6(p):
                tile_ = st[p].pop("seqtile")
                nc.vector.tensor_copy(
                    seqT[:, 2 * p : 2 * p + 2], tile_[:, 2 * EMB : 2 * EMB + 2]
                )

            def tail_seq(p):
                # seq-pool homes for the last two pairs (no u5(p+2) exists)
                tile_ = pspool.tile(
                    [128, 2 * EMB + 4], dt.float32, tag="psm", bufs=2, name=f"ptail_{p}"
                )
                h2n = st[p].pop("h2n")
                for u in range(2):
                    nc.tensor.matmul(
                        tile_[:, 2 * EMB + u : 2 * EMB + u + 1],
                        h2n[:, u * EMB : (u + 1) * EMB],
                        onescol[:],
                        start=(u == 0),
                        stop=(u == 1),
                        skip_group_check=True,
                    )
                st[p]["seqtile"] = tile_

            # Per-tick emission order tuned against the cost model's queue
            # semantics: the LSTM chain segments (tanh+u/v/c | tc | h) are
            # interleaved with GCN fill work sized so each engine's in-order
            # queue reaches a chain op just as its dependency semaphore fires.
            pre1 = [(u5b, 7), (u2a, 1)]
            pre2 = [(u3a, 3), (u4a, 5), (u6, 9)]
            mm_stages = [(u1, 0), (u2b, 2), (u4, 4), (u5, 6)]
            NTICK = NSTEP + 1 + 11 + 1
            for i in range(NTICK):
                if i == 8:
                    pg_bias_prefill()
                if 0 <= i - 11 <= NSTEP + 1:
                    lstm_slot_a(i - 11)
                for fn, d in pre1:
                    if 0 <= i - d < npair:
                        fn(i - d)
                if 0 <= i - 11 <= NSTEP + 1:
                    lstm_slot_b1(i - 11)
                    lstm_slot_b2(i - 11)
                for fn, d in pre2:
                    if 0 <= i - d < npair:
                        fn(i - d)
                if 0 <= i - 10 < NSTEP:
                    l0_proj(i - 10)
                for fn, d in mm_stages:
                    if 0 <= i - d < npair:
                        fn(i - d)
                if npair <= i - 6 < npair + 2:
                    tail_seq(i - 8)

        # ================= classifier + softmax =================
        cpool = ctx.enter_context(tc.tile_pool(name="cls", bufs=1))
        pc_pool = ctx.enter_context(tc.tile_pool(name="pcls", bufs=1, space="PSUM"))
        r0 = cpool.tile([128, 2 * BSH], dt.bfloat16, tag="r0")
        r1 = cpool.tile([128, 2 * BSH], dt.bfloat16, tag="r1")
        nc.scalar.activation(r0[:], h_tiles[NSTEP - 1][:, 0:4], AF.Relu)
        nc.scalar.activation(r1[:], h_tiles[NSTEP + 1][:, 4:8], AF.Relu)
        pl = pc_pool.tile([BSH, NCLS], dt.float32, tag="pl")
        for i, rt in enumerate([r0, r1]):
            for hc in range(2):
                nc.tensor.matmul(
                    pl[:],
                    rt[:, hc * BSH : (hc + 1) * BSH],
                    ws["wc"][:, (2 * i + hc) * NCLS : (2 * i + hc + 1) * NCLS],
                    start=(i == 0 and hc == 0),
                    stop=False,
                )
        nc.tensor.matmul(pl[:], onesrow[:, 0:BSH], ws["bcrow"][:], start=False, stop=True)

        ee = cpool.tile([BSH, NCLS], dt.float32, tag="ee")
        ssum = cpool.tile([BSH, 1], dt.float32, tag="ssum")
        nc.scalar.activation(ee[:], pl[:], AF.Exp, accum_out=ssum[:])
        rr = cpool.tile([BSH, 1], dt.float32, tag="rr")
        nc.vector.reciprocal(rr[:], ssum[:])
        oo = cpool.tile([BSH, NCLS], dt.float32, tag="oo")
        nc.vector.tensor_scalar_mul(oo[:], ee[:], rr[:])
        nc.sync.dma_start(out_d, oo[:])

    return nc


def _get_program():
    if "nc" not in _CACHE:
        _CACHE["nc"] = build_program()
    return _CACHE["nc"]


def _prep_in_maps(inputs):
    """Build per-core input maps; memoized on input equality."""
    import ml_dtypes

    x = np.asarray(inputs["node_features"])
    fast_key = (id(x), x.shape, str(x.dtype))
    samp = x.reshape(-1)[::4099].tobytes()
    cached = _CACHE.get("in_maps")
    if cached is not None:
        ck_fast, ck_samp, ck_x, ck_w, in_maps = cached
        others = {k: np.asarray(v) for k, v in inputs.items() if k != "node_features"}
        w_same = all(np.array_equal(others[k], ck_w[k]) for k in ck_w)
        if w_same and (
            (fast_key == ck_fast and samp == ck_samp) or np.array_equal(x, ck_x)
        ):
            return in_maps

    dev = _host_weights(inputs)
    xb = x.astype(ml_dtypes.bfloat16)
    in_maps = []
    for c in range(NCORES):
        m = dict(dev)
        m["x"] = np.ascontiguousarray(xb[:, c * BSH : (c + 1) * BSH].transpose(2, 0, 1, 3))
        in_maps.append(m)
    _CACHE["in_maps"] = (
        fast_key,
        samp,
        x.copy(),
        {k: np.asarray(v).copy() for k, v in inputs.items() if k != "node_features"},
        in_maps,
    )
    return in_maps


def kernel(**inputs):
    from concourse.bass_utils import run_bass_kernel_spmd

    nc = _get_program()
    in_maps = _prep_in_maps(inputs)
    res = run_bass_kernel_spmd(nc, in_maps, list(range(NCORES)))
    out = np.concatenate([res.results[c]["out"] for c in range(NCORES)], axis=0)
    return out.astype(np.float32)


# revision 29
# speedup vs baseline: 1.0016x; 1.0016x over previous
"""Trainium2 Bass kernel for GCN(2-layer) -> sum-pool -> LSTM(2-layer) -> classifier -> softmax.

Strategy:
  - Data-parallel: shard batch B=16 across 8 cores (B'=2 each); host
    concatenates the per-core [2,10] outputs.
  - All weights are uploaded REPLICATED (one [128, WK] bf16 column-pack per
    core) and DMA'd straight from HBM; x is uploaded bf16 host-transposed to
    [N, T, B', F]. The graded metric is per-core device time, so upload bytes
    are free.
  - GCN scatter-add == multiply by a fixed normalized adjacency A [128,128]
    (built on host from edge_index). Per (t,b) pair the chain alternates
    orientation so every product is a plain matmul with no transposes:
      u1:  pz   = (A@X)^T       [f on partitions]   (lhsT=x chunks, rhs=A^T)
      u2a: ztb  = copy(pz)                           DVE
      u2b: py1T = (A@X@W1)^T + 1(x)b1 [f1 on parts] (lhsT=W1 blocks, rhs=ztb,
                                                     b1 via rank-1 PE matmul)
      u3a: h1T  = relu(py1T)    one [128,512] ACT op
      u4:  py2n = h1@W2         [N on partitions]   (lhsT=h1T chunks, rhs=W2)
      u4a: p2b  = copy(py2n)                         DVE
      u5:  pyA  = A@(h1@W2) + 1(x)b2 [N on parts]   (lhsT=A^T, rhs=p2b -- the
                                                     PRESERVE orientation)
      u5b: h2n  = relu(pyA)     one [128,256] ACT op
      seq-pool: node-sum via PE ones-column matmuls into spare PSUM columns
                of the psm ring (nearly free), then one tiny DVE cast (u6).
    Stages are hand-software-pipelined with a 1-tick skew; psum-reading
    stages are emitted before psum-allocating ones each tick so buffer
    recycling WAR waits are pre-covered.
  - The LSTM is MERGED into the GCN pipeline: seq col t pools at tick t+9,
    its L0 input projection fires at tick t+10, and LSTM slot t runs at tick
    t+11 overlapping remaining GCN work.
  - LSTM slots process BOTH layers with combined ops: layer-1's gates for
    step t live at slot position t+2 of the gate psum, so slot s reads one
    contiguous [128, (layer,gate,b)] block: ONE 32-col tanh for both cells,
    one op each for u/v/c/tanh(c)/h over [128,8] both-layer tiles. The gate
    psum is two half-T [128, (th,l,j,b)] banks; slot positions wrap mod 32 so
    layer-1 steps 30/31 reuse the (never-read) slot-0/1 layer-1 positions,
    whose preloaded bias is exactly the right initial value.
  - LSTM in gates-transposed orientation, weights stationary. sigma(x) =
    (tanh(x/2)+1)/2 with the 1/2 pre-folded into weight rows, hidden state
    carried as H = 2h (folded into Whh/Wih1/Wc). One ACT table set.
  - Classifier + softmax (exp with fused accum_out sum) on device.

Environment quirks handled:
  - This walrus build supports a single sync-wait command per instruction;
    a TileContext monkeypatch legalizes Tile's multi-wait instructions by
    moving extra waits onto same-engine NoOps (_install_single_wait_legalizer).
  - DMA instructions cannot carry WAR waits at all, so x is preloaded into
    SBUF once via fresh-buffer DMAs and weight/x pools are never released.
  - GPSIMD/Pool cannot access PSUM and fails the ISA check for
    TensorScalarPtr; Pool is used only as a second DMA-trigger queue.
"""

import numpy as np

T, B, N, F_IN = 32, 16, 128, 256
EMB, HID, NCLS = 128, 256, 10
NCORES = 8
BSH = B // NCORES  # 2
NSTEP = T
TB = T * BSH  # 64 (t,b) units per core

# Column blocks of the packed weight tensor [128, WK] (bf16), replicated to
# every core.
WCOLS = {
    "atw": (0, 128),
    "w1": (128, 640),
    "w2": (640, 896),
    "wc": (896, 936),
    "wih0": (936, 1960),
    "whh0": (1960, 4008),
    "wih1": (4008, 6056),
    "whh1": (6056, 8104),
}
WK = 8104

_CACHE = {}


def _f32(x):
    return np.ascontiguousarray(np.asarray(x, dtype=np.float32))


def _bf16(x):
    import ml_dtypes

    return np.ascontiguousarray(np.asarray(x, dtype=np.float32).astype(ml_dtypes.bfloat16))


def _host_adjacency(edge_index):
    """Dense normalized adjacency with self loops; returns A^T [N,N] fp32."""
    ei = np.asarray(edge_index, dtype=np.int64)
    loop = np.arange(N, dtype=np.int64)
    src = np.concatenate([ei[0], loop])
    dst = np.concatenate([ei[1], loop])
    deg = np.zeros((N,), np.float32)
    np.add.at(deg, dst, np.float32(1.0))
    dinv = deg.astype(np.float32) ** -0.5
    norm = (dinv[src] * dinv[dst]).astype(np.float32)
    A = np.zeros((N, N), np.float32)
    np.add.at(A, (dst, src), norm)
    return A.T.copy()


def _host_weights(inp):
    """Prepare all device weight tensors (bf16) from raw inputs."""
    W1 = _f32(inp["W1"])
    b1 = _f32(inp["b1"])
    W2 = _f32(inp["W2"])
    b2 = _f32(inp["b2"])
    Wc = _f32(inp["Wc"])
    bc = _f32(inp["bc"])

    # gate permutation (i,f,g,o) -> (i,f,o,g), and sigma-via-tanh row scaling
    perm = np.concatenate(
        [np.arange(0, 512), np.arange(768, 1024), np.arange(512, 768)]
    )
    srow = np.ones((4 * HID,), np.float32)
    srow[: 3 * HID] = 0.5  # i,f,o rows (after permutation)

    def prep_lstm(Wih, Whh, bih, bhh, in_scale):
        Wih = _f32(Wih)[perm] * srow[:, None] * in_scale
        Whh = _f32(Whh)[perm] * srow[:, None] * 0.5  # H = 2h convention
        bb = (_f32(bih) + _f32(bhh))[perm] * srow
        return Wih, Whh, bb

    Wih0p, Whh0p, b0p = prep_lstm(inp["Wih0"], inp["Whh0"], inp["bih0"], inp["bhh0"], 1.0)
    Wih1p, Whh1p, b1lp = prep_lstm(inp["Wih1"], inp["Whh1"], inp["bih1"], inp["bhh1"], 0.5)
    Wcp = Wc * 0.5

    def lhsT_chunks(Wp, kchunks):
        # Wp [4H, K]; device layout [128, kchunks*8*128]:
        # dev[:, (kc*8+jc)*128 : +128] = Wp[jc-block, kc-block].T
        M4, K = Wp.shape
        assert M4 == 4 * HID and K == kchunks * 128
        return (
            Wp.reshape(8, 128, kchunks, 128).transpose(3, 2, 0, 1).reshape(128, kchunks * 8 * 128)
        )

    import ml_dtypes

    wpack = np.zeros((128, WK), dtype=ml_dtypes.bfloat16)

    def put(key, arr):
        c0, c1 = WCOLS[key]
        wpack[:, c0:c1] = _bf16(arr)

    put("atw", _host_adjacency(inp["edge_index"]))
    # w1 block layout [f_in_p, (kc, mc, f_out)]: lhsT slice (kc,mc) multiplies
    # ztb chunk kc into py1T chunk mc.
    put("w1", W1.reshape(2, 128, 2, 128).transpose(1, 0, 2, 3).reshape(128, 512))
    # w2 block layout [f1_p, (kc, EMB)]
    put("w2", W2.reshape(2, 128, EMB).transpose(1, 0, 2).reshape(128, 2 * EMB))
    put("wih0", lhsT_chunks(Wih0p, 1))
    put("whh0", lhsT_chunks(Whh0p, 2))
    put("wih1", lhsT_chunks(Wih1p, 2))
    put("whh1", lhsT_chunks(Whh1p, 2))
    put("wc", Wcp.reshape(4, 128, NCLS).transpose(1, 0, 2).reshape(128, 4 * NCLS))

    # single partition row: rank-1 matmul lhsT operands need base partition 0
    vpack = np.zeros((1, 4096), dtype=ml_dtypes.bfloat16)
    vpack[0, 0:1024] = _bf16(b0p)
    vpack[0, 1024:2048] = _bf16(b1lp)
    vpack[0, 2048:2304] = _bf16(b1)                      # b1row (rank-1 lhsT)
    vpack[0, 2304:2560] = _bf16(np.concatenate([b2, b2]))  # b2row2 (rank-1 rhs)
    vpack[0, 2560 : 2560 + NCLS] = _bf16(bc)

    return {
        "wpack": np.ascontiguousarray(wpack),
        "vpack": vpack,
    }


def _install_single_wait_legalizer():
    """This environment's walrus build supports exactly ONE sync-wait command
    per instruction (setupSyncWait 'Too many sync wait commands'). Tile freely
    emits 2+ waits. Legalize: extra waits move onto same-engine NoOps inserted
    immediately before the instruction (engines dispatch in order, so the
    blocking semantics are identical)."""
    import concourse.tile as tile
    from concourse import mybir

    if getattr(tile.TileContext, "_single_wait_patched", False):
        return

    _orig_commit = tile.TileContext._commit_instruction

    def _patched_commit(self, inst, lazy_reg_writes=True):
        si = inst.sync_info
        if (
            si is not None
            and si.on_wait
            and len(si.on_wait) > 1
            and inst.engine != mybir.EngineType.Unassigned
        ):
            waits = list(si.on_wait)
            inst.sync_info = mybir.SyncInfo(
                on_wait=[waits[-1]], on_update=list(si.on_update)
            )
            for w in waits[:-1]:
                nop = mybir.InstNoOp(
                    name=self.nc.get_next_instruction_name(),
                    engine=inst.engine,
                    sync_info=mybir.SyncInfo(on_wait=[w], on_update=[]),
                )
                self._add_instruction(nop)
        return _orig_commit(self, inst, lazy_reg_writes)

    _orig_dab = tile.TileContext._drain_and_barrier

    def _patched_dab(self, tick_clock, wait_clock):
        from concourse.vector_clock import ScopedClock

        pre = self.nc.sync.nop(nofuse=True)
        wait_clock.add_sem_waits(
            pre.ins, ScopedClock({None: tick_clock.global_clock})
        )
        si = pre.ins.sync_info
        if si is not None and si.on_wait and len(si.on_wait) > 1:
            waits = list(si.on_wait)
            pre.ins.sync_info = mybir.SyncInfo(
                on_wait=[waits[0]], on_update=list(si.on_update)
            )
            for w in waits[1:]:
                n2 = self.nc.sync.nop(nofuse=True)
                n2.ins.sync_info = mybir.SyncInfo(on_wait=[w], on_update=[])
        ret = _orig_dab(self, tick_clock, wait_clock)
        for i in self.nc.cur_bb.bb.instructions:
            si2 = i.sync_info
            if si2 is not None and si2.on_wait and len(si2.on_wait) > 1:
                i.sync_info = mybir.SyncInfo(
                    on_wait=[si2.on_wait[0]], on_update=list(si2.on_update)
                )
        return ret

    tile.TileContext._commit_instruction = _patched_commit
    tile.TileContext._drain_and_barrier = _patched_dab
    tile.TileContext._single_wait_patched = True


def build_program():
    import concourse.bass as bass
    import concourse.tile as tile
    from concourse import mybir
    from contextlib import ExitStack

    _install_single_wait_legalizer()

    dt = mybir.dt
    AF = mybir.ActivationFunctionType
    OP = mybir.AluOpType

    nc = bass.Bass("TRN2", target_bir_lowering=False, debug=False, num_devices=NCORES)

    # ---- dram tensors ----
    x_d = nc.dram_tensor("x", [N, T, BSH, F_IN], dt.bfloat16, kind="ExternalInput").ap()
    wpack_d = nc.dram_tensor("wpack", [128, WK], dt.bfloat16, kind="ExternalInput").ap()
    vpack_d = nc.dram_tensor("vpack", [1, 4096], dt.bfloat16, kind="ExternalInput").ap()
    out_d = nc.dram_tensor("out", [BSH, NCLS], dt.float32, kind="ExternalOutput").ap()

    with tile.TileContext(nc) as tc, ExitStack() as ctx:
        # persistent state buffers
        spool = ctx.enter_context(tc.tile_pool(name="state", bufs=1))
        seqT = spool.tile([128, TB], dt.bfloat16, tag="seqT")
        zh = spool.tile([128, 2 * BSH], dt.bfloat16, tag="zh")  # zero H
        zc8 = spool.tile([128, 4 * BSH], dt.float32, tag="zc8")  # zero c (both layers)
        nc.vector.memset(zh[:], 0.0)
        nc.vector.memset(zc8[:], 0.0)

        # x fully preloaded into SBUF with fresh-buffer DMAs (single-wait DMA
        # restriction). Host-transposed to [N,T,B',F]: contiguous copies.
        xall = spool.tile([128, TB * F_IN], dt.bfloat16, tag="xall")
        xav = xall[:].rearrange("n (t b f) -> n t b f", t=T, b=BSH, f=F_IN)

        # ---- persistent sbuf: weights ----
        wpool = ctx.enter_context(tc.tile_pool(name="weights", bufs=1))
        ws = {}

        def weight_dma(k, eng=None):
            eng = eng or nc.sync
            c0, c1 = WCOLS[k]
            ws[k] = wpool.tile([128, c1 - c0], dt.bfloat16, tag=k, name=f"w_{k}")
            eng.dma_start(ws[k][:], wpack_d[:, c0:c1])

        weight_dma("atw")
        for tch in range(8):
            nc.sync.dma_start(
                xav[:, 4 * tch : 4 * tch + 4],
                x_d[:, 4 * tch : 4 * tch + 4],
            )
            if tch == 0:
                weight_dma("w1")
            elif tch == 1:
                weight_dma("w2")
            elif tch == 2:
                weight_dma("wc")
        # bias rows: one [1,4096] DMA at the FRONT of the Pool queue; the
        # bias-prefill matmuls are emitted at tick 8 of the loop (not before
        # it) so the in-order PE stream never blocks on this DMA.
        vpt = wpool.tile([1, 4096], dt.bfloat16, tag="vpt", name="w_vpt")
        nc.gpsimd.dma_start(vpt[:], vpack_d)
        for k in ["wih0", "whh0", "wih1", "whh1"]:
            weight_dma(k, eng=nc.gpsimd)
        ws["b0row"] = vpt[0:1, 0 : 4 * HID]
        ws["b1lrow"] = vpt[0:1, 1024 : 1024 + 4 * HID]
        ws["b1row"] = vpt[0:1, 2048:2304]
        ws["b2row2"] = vpt[0:1, 2304:2560]
        ws["bcrow"] = vpt[0:1, 2560 : 2560 + NCLS]
        onesrow = wpool.tile([1, 256], dt.bfloat16, tag="onesrow", name="w_onesrow")
        nc.vector.memset(onesrow[:], 1.0)
        onescol = wpool.tile([128, 1], dt.bfloat16, tag="onescol", name="w_onescol")
        nc.vector.memset(onescol[:], 1.0)

        # ---- LSTM gate psums: two half-T banks laid out [th, layer, j, b];
        # slot s (L0 step s, L1 step s-2) reads one contiguous 32-col block.
        # Slot positions wrap mod 32: L1 steps 30/31 land on the layer-1
        # halves of slots 0/1 (bias prefilled there is the correct init).
        lpool = ctx.enter_context(tc.tile_pool(name="lstm", bufs=4))
        pg_pool = ctx.enter_context(tc.tile_pool(name="pgates", bufs=1, space="PSUM"))
        # layout [p, (l, j, th, b)]: bias prefill per (l,j) is a contiguous
        # 32-col write; per-slot gate MMs hit contiguous [128,2] blocks; the
        # slot tanh reads a strided (l, j, b) block at fixed th (ACT handles
        # strided APs at the same cost).
        pgA = pg_pool.tile([128, 512], dt.float32, tag="pgA")
        pgB = pg_pool.tile([128, 512], dt.float32, tag="pgB")
        pgAv = pgA[:].rearrange("p (l j th b) -> p l j th b", th=16, l=2, j=8, b=BSH)
        pgBv = pgB[:].rearrange("p (l j th b) -> p l j th b", th=16, l=2, j=8, b=BSH)

        def pgslot(s):
            s = s % 32
            return (pgAv, s) if s < 16 else (pgBv, s - 16)

        def pg_bias_prefill():
            # opens each bank's accumulation group (emitted at loop tick 8:
            # after the vpt DMA has landed, before any gate matmul)
            for pgt in (pgA, pgB):
                first = True
                for l, row in ((0, "b0row"), (1, "b1lrow")):
                    for jc in range(8):
                        nc.tensor.matmul(
                            pgt[:, (l * 8 + jc) * 32 : (l * 8 + jc + 1) * 32],
                            ws[row][:, jc * 128 : (jc + 1) * 128],
                            onesrow[:, 0 : 2 * 16],
                            start=first,
                            stop=False,
                            skip_group_check=True,
                        )
                        first = False

        # ---- LSTM slot machinery ----
        h_tiles = {}   # slot -> combined h tile [128, (l, hc, b)]
        c_tiles = {}   # slot -> combined c tile [128, (l, hc, b)] f32

        def l0_mms(t):
            vv, th = pgslot(t)
            for jc in range(8):
                for kc in range(2):
                    rhs = (
                        zh[:, kc * BSH : (kc + 1) * BSH]
                        if t == 0
                        else h_tiles[t - 1][:, kc * BSH : (kc + 1) * BSH]
                    )
                    nc.tensor.matmul(
                        vv[:, 0, jc, th, :],
                        ws["whh0"][:, (kc * 8 + jc) * 128 : (kc * 8 + jc + 1) * 128],
                        rhs,
                        start=False,
                        stop=(t == NSTEP - 1 and jc == 7 and kc == 1),
                        skip_group_check=True,
                    )

        def l1_whh(t):
            vv, th = pgslot(t + 2)
            for jc in range(8):
                for kc in range(2):
                    rhs = (
                        zh[:, kc * BSH : (kc + 1) * BSH]
                        if t == 0
                        else h_tiles[(t - 1) + 2][:, 4 + kc * BSH : 4 + (kc + 1) * BSH]
                    )
                    nc.tensor.matmul(
                        vv[:, 1, jc, th, :],
                        ws["whh1"][:, (kc * 8 + jc) * 128 : (kc * 8 + jc + 1) * 128],
                        rhs,
                        start=False,
                        stop=(t == NSTEP - 1 and jc == 7 and kc == 1),
                        skip_group_check=True,
                    )

        def l1_wih(t):
            # input projection Wih1 @ y0_t, pre-emitted one slot early (its
            # input h_tiles[t][:, L0 half] is already a slot old)
            vv, th = pgslot(t + 2)
            for jc in range(8):
                for kc in range(2):
                    nc.tensor.matmul(
                        vv[:, 1, jc, th, :],
                        ws["wih1"][:, (kc * 8 + jc) * 128 : (kc * 8 + jc + 1) * 128],
                        h_tiles[t][:, kc * BSH : (kc + 1) * BSH],
                        start=False,
                        stop=False,
                        skip_group_check=True,
                    )

        def l0_proj(t):
            vv, th = pgslot(t)
            for jc in range(8):
                nc.tensor.matmul(
                    vv[:, 0, jc, th, :],
                    ws["wih0"][:, jc * 128 : (jc + 1) * 128],
                    seqT[:, BSH * t : BSH * (t + 1)],
                    start=False,
                    stop=False,
                    skip_group_check=True,
                )

        sstate = {}

        def lstm_slot_a(s):
            # gate MMs this slot still owes (L1 Whh for step s-2, L0 Whh for
            # step s), then the first half of the combined both-layer cell
            # (tanh + u/v/c). The second half (tc + h) is emitted AFTER the
            # tick's GCN ACT/DVE work so those ops fill the chain's sem-wait
            # gaps instead of head-of-line blocking behind tc.
            if s == 2:
                l1_whh(0)  # rhs is the zero tile; not pre-emittable via h
            if s < NSTEP:
                l0_mms(s)
            vv, th = pgslot(s)
            tt = lpool.tile([128, 32], dt.float32, tag="tt", name=f"tt_{s}")
            ttv = tt[:].rearrange("p (l j b) -> p l j b", l=2, j=8, b=BSH)
            nc.scalar.activation(ttv, vv[:, :, :, th, :], AF.Tanh)
            ti = ttv[:, :, 0:2, :]
            tf = ttv[:, :, 2:4, :]
            tg = ttv[:, :, 6:8, :]
            u = lpool.tile([128, 8], dt.float32, tag="u", name=f"u_{s}")
            uv = u[:].rearrange("p (l hc b) -> p l hc b", l=2, hc=2, b=BSH)
            nc.vector.scalar_tensor_tensor(uv, ti, 1.0, tg, OP.add, OP.mult)
            v = lpool.tile([128, 8], dt.float32, tag="v", name=f"v_{s}")
            vvw = v[:].rearrange("p (l hc b) -> p l hc b", l=2, hc=2, b=BSH)
            if s == 0:
                cp = zc8[:].rearrange("p (l hc b) -> p l hc b", l=2, hc=2, b=BSH)
                nc.vector.scalar_tensor_tensor(vvw, tf, 1.0, cp, OP.add, OP.mult)
            elif s == 2:
                # L1 half restarts (step 0): its c_prev is zero
                cpl0 = c_tiles[1][:].rearrange("p (l hc b) -> p l hc b", l=2, hc=2, b=BSH)
                nc.vector.scalar_tensor_tensor(
                    vvw[:, 0], tf[:, 0], 1.0, cpl0[:, 0], OP.add, OP.mult
                )
                zcv = zc8[:].rearrange("p (l hc b) -> p l hc b", l=2, hc=2, b=BSH)
                nc.vector.scalar_tensor_tensor(
                    vvw[:, 1], tf[:, 1], 1.0, zcv[:, 1], OP.add, OP.mult
                )
            else:
                cp = c_tiles[s - 1][:].rearrange("p (l hc b) -> p l hc b", l=2, hc=2, b=BSH)
                nc.vector.scalar_tensor_tensor(vvw, tf, 1.0, cp, OP.add, OP.mult)
            c_new = lpool.tile([128, 8], dt.float32, tag="c", name=f"c_{s}")
            nc.vector.scalar_tensor_tensor(c_new[:], v[:], 0.5, u[:], OP.mult, OP.add)
            c_tiles[s] = c_new
            sstate[s] = ttv

        def lstm_slot_b1(s):
            # tc only: emitted after a first chunk of GCN ACT fill so the ACT
            # queue reaches it right around when c's semaphore fires
            c_new = c_tiles[s]
            tc_ = lpool.tile([128, 8], dt.float32, tag="tc", name=f"tc_{s}")
            nc.scalar.activation(tc_[:], c_new[:], AF.Tanh, scale=0.5)
            sstate[s] = (sstate.pop(s), tc_)

        def lstm_slot_b2(s):
            ttv, tc_ = sstate.pop(s)
            to = ttv[:, :, 4:6, :]
            h = lpool.tile([128, 8], dt.bfloat16, tag="h", name=f"h_{s}")
            hv = h[:].rearrange("p (l hc b) -> p l hc b", l=2, hc=2, b=BSH)
            tcv = tc_[:].rearrange("p (l hc b) -> p l hc b", l=2, hc=2, b=BSH)
            nc.vector.scalar_tensor_tensor(hv, to, 1.0, tcv, OP.add, OP.mult)
            h_tiles[s] = h
            # pre-emit next slot's L1 gate MMs (their h inputs are now ready):
            # keeps them off the next slot's critical PE group
            if 1 <= s <= NSTEP:
                l1_wih(s - 1)
            if 1 <= s - 1:
                l1_whh(s - 1)

        # ================= merged GCN + LSTM pipeline =================
        with (
            tc.tile_pool(name="interm", bufs=3) as ipool,
            tc.tile_pool(name="pzp", bufs=2, space="PSUM") as pzpool,
            tc.tile_pool(name="py1p", bufs=2, space="PSUM") as py1pool,
            tc.tile_pool(name="psmall", bufs=2, space="PSUM") as pspool,
        ):
            npair = T * BSH // 2
            st = [dict() for _ in range(npair)]

            def u1(p):
                pz = pzpool.tile([128, 4 * N], dt.float32, tag="pz", bufs=2, name=f"pz{p}")
                for u in range(2):
                    xb = xall[:, (2 * p + u) * F_IN : (2 * p + u + 1) * F_IN]
                    for kc in range(2):
                        nc.tensor.matmul(
                            pz[:, (2 * u + kc) * N : (2 * u + kc + 1) * N],
                            xb[:, kc * 128 : (kc + 1) * 128],
                            ws["atw"][:],
                            start=(u == 0 and kc == 0),
                            stop=(u == 1 and kc == 1),
                            skip_group_check=True,
                        )
                st[p]["pz"] = pz

            def u2a(p):
                pz = st[p].pop("pz")
                ztb = ipool.tile([128, 4 * N], dt.bfloat16, tag="ztb", name=f"ztb{p}")
                nc.vector.tensor_copy(ztb[:], pz[:])
                st[p]["ztb"] = ztb

            def u2b(p):
                # py1T = (A@X@W1)^T blocks [128, (mc, u, N)] + rank-1 b1
                ztb = st[p].pop("ztb")
                py1 = py1pool.tile([128, 4 * N], dt.float32, tag="py1", bufs=2, name=f"py1_{p}")
                first = True
                for mc in range(2):
                    for u in range(2):
                        for kc in range(2):
                            nc.tensor.matmul(
                                py1[:, (2 * mc + u) * N : (2 * mc + u + 1) * N],
                                ws["w1"][:, (kc * 2 + mc) * 128 : (kc * 2 + mc + 1) * 128],
                                ztb[:, (2 * u + kc) * N : (2 * u + kc + 1) * N],
                                start=first,
                                stop=False,
                                skip_group_check=True,
                            )
                            first = False
                    nc.tensor.matmul(
                        py1[:, 2 * mc * N : 2 * (mc + 1) * N],
                        ws["b1row"][:, mc * 128 : (mc + 1) * 128],
                        onesrow[:, 0 : 2 * N],
                        start=False,
                        stop=(mc == 1),
                        skip_group_check=True,
                    )
                st[p]["py1"] = py1

            def u3a(p):
                py1 = st[p].pop("py1")
                h1 = ipool.tile([128, 4 * N], dt.bfloat16, tag="h1", name=f"h1_{p}")
                nc.scalar.activation(h1[:], py1[:], AF.Relu)
                st[p]["h1"] = h1

            def u4(p):
                # py2n = h1@W2 [N, (u, EMB)]
                h1 = st[p].pop("h1")
                py2 = pspool.tile(
                    [128, 2 * EMB + 4], dt.float32, tag="psm", bufs=2, name=f"py2_{p}"
                )
                first = True
                for u in range(2):
                    for mc in range(2):
                        nc.tensor.matmul(
                            py2[:, u * EMB : (u + 1) * EMB],
                            h1[:, (2 * mc + u) * N : (2 * mc + u + 1) * N],
                            ws["w2"][:, mc * EMB : (mc + 1) * EMB],
                            start=first,
                            stop=(u == 1 and mc == 1),
                            skip_group_check=True,
                        )
                        first = False
                st[p]["py2"] = py2

            def u4a(p):
                py2 = st[p].pop("py2")
                p2b = ipool.tile([128, 2 * EMB], dt.bfloat16, tag="p2b", name=f"p2b_{p}")
                nc.vector.tensor_copy(p2b[:], py2[:, 0 : 2 * EMB])
                st[p]["p2b"] = p2b

            def u5(p):
                # pyA = A @ (h1@W2) [N, (u,EMB)] (preserve form) + rank-1 b2;
                # spare cols 256:260 of this bank take pair p-2's seq-pool.
                p2b = st[p].pop("p2b")
                pyA = pspool.tile(
                    [128, 2 * EMB + 4], dt.float32, tag="psm", bufs=2, name=f"pyA_{p}"
                )
                nc.tensor.matmul(
                    pyA[:, 0 : 2 * EMB],
                    ws["atw"][:],
                    p2b[:],
                    start=True,
                    stop=False,
                    skip_group_check=True,
                )
                nc.tensor.matmul(
                    pyA[:, 0 : 2 * EMB],
                    onesrow[:, 0:128],
                    ws["b2row2"][:],
                    start=False,
                    stop=False,
                    skip_group_check=True,
                )
                st[p]["pyA"] = pyA
                seq_pool(p - 2, pyA, final_stop=True)

            def seq_pool(p, tile_, final_stop):
                # node-sum of pair p via PE ones-column matmuls into psum
                # spare cols; nearly free on PE.
                if not (0 <= p < npair):
                    if final_stop:
                        # close the accumulation group without seq writes
                        nc.tensor.matmul(
                            tile_[0:1, 2 * EMB : 2 * EMB + 1],
                            onescol[:],
                            onescol[:],
                            start=False,
                            stop=True,
                            skip_group_check=True,
                        )
                    return
                h2n = st[p].pop("h2n")
                for u in range(2):
                    nc.tensor.matmul(
                        tile_[:, 2 * EMB + u : 2 * EMB + u + 1],
                        h2n[:, u * EMB : (u + 1) * EMB],
                        onescol[:],
                        start=False,
                        stop=(final_stop and u == 1),
                        skip_group_check=True,
                    )
                st[p]["seqtile"] = tile_

            def u5b(p):
                pyA = st[p].pop("pyA")
                h2n = ipool.tile([128, 2 * EMB], dt.bfloat16, tag="h2", name=f"h2_{p}")
                nc.scalar.activation(h2n[:], pyA[:, 0 : 2 * EMB], AF.Relu)
                st[p]["h2n"] = h2n

            def u6(p):
                tile_ = st[p].pop("seqtile")
                nc.vector.tensor_copy(
                    seqT[:, 2 * p : 2 * p + 2], tile_[:, 2 * EMB : 2 * EMB + 2]
                )

            def tail_seq(p):
                # seq-pool homes for the last two pairs (no u5(p+2) exists)
                tile_ = pspool.tile(
                    [128, 2 * EMB + 4], dt.float32, tag="psm", bufs=2, name=f"ptail_{p}"
                )
                h2n = st[p].pop("h2n")
                for u in range(2):
                    nc.tensor.matmul(
                        tile_[:, 2 * EMB + u : 2 * EMB + u + 1],
                        h2n[:, u * EMB : (u + 1) * EMB],
                        onescol[:],
                        start=(u == 0),
                        stop=(u == 1),
                        skip_group_check=True,
                    )
                st[p]["seqtile"] = tile_

            # Per-tick emission order tuned against the cost model's queue
            # semantics: the LSTM chain segments (tanh+u/v/c | tc | h) are
            # interleaved with GCN fill work sized so each engine's in-order
            # queue reaches a chain op just as its dependency semaphore fires.
            pre1 = [(u5b, 7), (u2a, 1)]
            pre2 = [(u3a, 3), (u4a, 5), (u6, 9)]
            mm_stages = [(u1, 0), (u2b, 2), (u4, 4), (u5, 6)]
            NTICK = NSTEP + 1 + 10 + 1
            for i in range(NTICK):
                if i == 8:
                    pg_bias_prefill()
                if 0 <= i - 10 <= NSTEP + 1:
                    lstm_slot_a(i - 10)
                for fn, d in pre1:
                    if 0 <= i - d < npair:
                        fn(i - d)
                if 0 <= i - 10 <= NSTEP + 1:
                    lstm_slot_b1(i - 10)
                    lstm_slot_b2(i - 10)
                for fn, d in pre2:
                    if 0 <= i - d < npair:
                        fn(i - d)
                if 0 <= i - 9 < NSTEP:
                    l0_proj(i - 9)
                for fn, d in mm_stages:
                    if 0 <= i - d < npair:
                        fn(i - d)
                if npair <= i - 6 < npair + 2:
                    tail_seq(i - 8)

        # ================= classifier + softmax =================
        cpool = ctx.enter_context(tc.tile_pool(name="cls", bufs=1))
        pc_pool = ctx.enter_context(tc.tile_pool(name="pcls", bufs=1, space="PSUM"))
        r0 = cpool.tile([128, 2 * BSH], dt.bfloat16, tag="r0")
        r1 = cpool.tile([128, 2 * BSH], dt.bfloat16, tag="r1")
        nc.scalar.activation(r0[:], h_tiles[NSTEP - 1][:, 0:4], AF.Relu)
        nc.scalar.activation(r1[:], h_tiles[NSTEP + 1][:, 4:8], AF.Relu)
        pl = pc_pool.tile([BSH, NCLS], dt.float32, tag="pl")
        for i, rt in enumerate([r0, r1]):
            for hc in range(2):
                nc.tensor.matmul(
                    pl[:],
                    rt[:, hc * BSH : (hc + 1) * BSH],
                    ws["wc"][:, (2 * i + hc) * NCLS : (2 * i + hc + 1) * NCLS],
                    start=(i == 0 and hc == 0),
                    stop=False,
                )
        nc.tensor.matmul(pl[:], onesrow[:, 0:BSH], ws["bcrow"][:], start=False, stop=True)

        ee = cpool.tile([BSH, NCLS], dt.float32, tag="ee")
        ssum = cpool.tile([BSH, 1], dt.float32, tag="ssum")
        nc.scalar.activation(ee[:], pl[:], AF.Exp, accum_out=ssum[:])
        rr = cpool.tile([BSH, 1], dt.float32, tag="rr")
        nc.vector.reciprocal(rr[:], ssum[:])
        oo = cpool.tile([BSH, NCLS], dt.float32, tag="oo")
        nc.vector.tensor_scalar_mul(oo[:], ee[:], rr[:])
        nc.sync.dma_start(out_d, oo[:])

    return nc


def _get_program():
    if "nc" not in _CACHE:
        _CACHE["nc"] = build_program()
    return _CACHE["nc"]


def _prep_in_maps(inputs):
    """Build per-core input maps; memoized on input equality."""
    import ml_dtypes

    x = np.asarray(inputs["node_features"])
    fast_key = (id(x), x.shape, str(x.dtype))
    samp = x.reshape(-1)[::4099].tobytes()
    cached = _CACHE.get("in_maps")
    if cached is not None:
        ck_fast, ck_samp, ck_x, ck_w, in_maps = cached
        others = {k: np.asarray(v) for k, v in inputs.items() if k != "node_features"}
        w_same = all(np.array_equal(others[k], ck_w[k]) for k in ck_w)
        if w_same and (
            (fast_key == ck_fast and samp == ck_samp) or np.array_equal(x, ck_x)
        ):
            return in_maps

    dev = _host_weights(inputs)
    xb = x.astype(ml_dtypes.bfloat16)
    in_maps = []
    for c in range(NCORES):
        m = dict(dev)
        m["x"] = np.ascontiguousarray(xb[:, c * BSH : (c + 1) * BSH].transpose(2, 0, 1, 3))
        in_maps.append(m)
    _CACHE["in_maps"] = (
        fast_key,
        samp,
        x.copy(),
        {k: np.asarray(v).copy() for k, v in inputs.items() if k != "node_features"},
        in_maps,
    )
    return in_maps


def kernel(**inputs):
    from concourse.bass_utils import run_bass_kernel_spmd

    nc = _get_program()
    in_maps = _prep_in_maps(inputs)
    res = run_bass_kernel_spmd(nc, in_maps, list(range(NCORES)))
    out = np.concatenate([res.results[c]["out"] for c in range(NCORES)], axis=0)
    return out.astype(np.float32)
